# revision 1
# baseline (speedup 1.0000x reference)
"""Trainium2 Bass kernel for the 4-directional Mamba (SS2D / VMamba-style)
block from the OSS reference.

Sharding: the 8 independent (direction x batch) sequences map one-per-core
(SPMD: one NEFF, 8 cores, per-core inputs). Backward directions are handled by
host-side flips of the input/output sequences; the final sum of the four
directional outputs plus the residual x2 happens at gather time on host.

Per-core kernel (C=96, L=4096, P=192, N=16, dtr=6):
  - causal depthwise conv folded into the input projection as 4 shifted
    tap-matmuls accumulating in PSUM (PE, fp16 operands)
  - silu/softplus built from the Exp/Ln ACT table only (2 table loads total)
  - dA_n = Exp(A[:, n] * dt) via ACT per-partition scale columns
  - the selective scan runs on the native DVE tensor_tensor_scan; the P=192
    state rows are packed as a full 128-row group plus a pair-packed 64-row
    group (two consecutive n per tile) so every per-n instruction uses all
    128 partitions; 24 scan items per L-section instead of 32
  - B/C rows round-trip through DRAM so one DMA partition-broadcasts both
    the B and C row for an item; sum over n runs on the PE as identity /
    selection matmuls accumulating in PSUM
  - elementwise work is split across DVE (scan, dBx, gates) / Pool (hc,
    silu pieces) / ACT (exp, psum moves) to balance engine busy time
"""

import numpy as np

C = 96
L = 4096
P = 192
PLO = 128
PHI = 64
N = 16
DTR = 6
DC = 4
HH = 64
WW = 64
MCH = 512
NSEC = 4
SEC = L // NSEC

_CACHED = {}


def _build_program(repeat=1, pool_hc=True, pool_c=True, n_bufs=4):
    from contextlib import ExitStack

    import concourse.bacc as bacc
    import concourse.bass as bass
    import concourse.tile as tile
    from concourse import mybir

    f32 = mybir.dt.float32
    f16 = mybir.dt.float16
    Alu = mybir.AluOpType
    Act = mybir.ActivationFunctionType

    nc = bacc.Bacc()

    seqT = nc.dram_tensor("seqT", [C, L], f16, kind="ExternalInput")
    wc = nc.dram_tensor("wc", [C, DC, P], f16, kind="ExternalInput")
    wz = nc.dram_tensor("wz", [C, P], f16, kind="ExternalInput")
    wxT = nc.dram_tensor("wxT", [P, DTR + 2 * N], f16, kind="ExternalInput")
    wdtT = nc.dram_tensor("wdtT", [DTR, P], f16, kind="ExternalInput")
    woT = nc.dram_tensor("woT", [P, C], f16, kind="ExternalInput")
    idsel = nc.dram_tensor("idsel", [PLO, PLO + PHI], f16, kind="ExternalInput")
    bdt = nc.dram_tensor("bdt", [P, 1], f32, kind="ExternalInput")
    cb = nc.dram_tensor("cb", [P, 1], f32, kind="ExternalInput")
    cbn = nc.dram_tensor("cbn", [P, 1], f32, kind="ExternalInput")
    dpv = nc.dram_tensor("dpv", [P, 1], f32, kind="ExternalInput")
    acol = nc.dram_tensor("acol", [P, N], f32, kind="ExternalInput")
    out = nc.dram_tensor("out", [C, L], f32, kind="ExternalOutput")

    with tile.TileContext(nc) as tc, ExitStack() as ctx:
        wpool = ctx.enter_context(tc.tile_pool(name="weights", bufs=1))
        spool = ctx.enter_context(tc.tile_pool(name="seq", bufs=1))
        big_pool = ctx.enter_context(tc.tile_pool(name="big", bufs=1))
        tmp_pool = ctx.enter_context(tc.tile_pool(name="tmp", bufs=2))
        n_pool = ctx.enter_context(tc.tile_pool(name="nl", bufs=n_bufs))
        bc_pool = ctx.enter_context(tc.tile_pool(name="bc", bufs=n_bufs))
        ps_pool = ctx.enter_context(tc.tile_pool(name="ps", bufs=3, space="PSUM"))
        ya_ps_pool = ctx.enter_context(
            tc.tile_pool(name="yps", bufs=1, space="PSUM"))
        dram_pool = ctx.enter_context(tc.tile_pool(name="dr", bufs=1, space="DRAM"))

        # ---- weights (lo = p 0:128, hi = p 128:192) ----
        t_wc = wpool.tile([C, DC, P], f16)
        t_wz = wpool.tile([C, P], f16)
        t_wxT = [wpool.tile([PLO, DTR + 2 * N], f16, name="wxT0"),
                 wpool.tile([PHI, DTR + 2 * N], f16, name="wxT1")]
        t_wdtT = wpool.tile([DTR, P], f16)
        t_woT = [wpool.tile([PLO, C], f16, name="woT0"),
                 wpool.tile([PHI, C], f16, name="woT1")]
        t_idsel = wpool.tile([PLO, PLO + PHI], f16)
        t_bdt = [wpool.tile([PLO, 1], f32, name="bdt0"),
                 wpool.tile([PHI, 1], f32, name="bdt1")]
        t_cb = [wpool.tile([PLO, 1], f32, name="cb0"),
                wpool.tile([PHI, 1], f32, name="cb1")]
        t_cbn = [wpool.tile([PLO, 1], f32, name="cbn0"),
                 wpool.tile([PHI, 1], f32, name="cbn1")]
        t_dpv = [wpool.tile([PLO, 1], f32, name="dpv0"),
                 wpool.tile([PHI, 1], f32, name="dpv1")]
        t_aclo = wpool.tile([PLO, N], f32)
        t_achi = wpool.tile([PLO, N // 2], f32)

        nc.sync.dma_start(out=t_wc, in_=wc[...])
        nc.sync.dma_start(out=t_wz, in_=wz[...])
        nc.sync.dma_start(out=t_wdtT, in_=wdtT[...])
        nc.sync.dma_start(out=t_idsel, in_=idsel[...])
        for i, (a, b) in enumerate([(0, PLO), (PLO, P)]):
            nc.sync.dma_start(out=t_wxT[i], in_=wxT[a:b, :])
            nc.sync.dma_start(out=t_woT[i], in_=woT[a:b, :])
            nc.sync.dma_start(out=t_bdt[i], in_=bdt[a:b, :])
            nc.sync.dma_start(out=t_cb[i], in_=cb[a:b, :])
            nc.sync.dma_start(out=t_cbn[i], in_=cbn[a:b, :])
            nc.sync.dma_start(out=t_dpv[i], in_=dpv[a:b, :])
        nc.sync.dma_start(out=t_aclo, in_=acol[0:PLO, :])
        # hi pair-packed A columns: col j = [A[128:192, 2j]; A[128:192, 2j+1]]
        hi = acol[PLO:P, :]
        nc.sync.dma_start(
            out=t_achi[0:PHI, :],
            in_=bass.AP(tensor=hi.tensor, offset=hi.offset,
                        ap=[list(hi.ap[0]), [2, N // 2]]))
        hi1 = acol[PLO:P, 1:]
        nc.sync.dma_start(
            out=t_achi[PHI:PLO, :],
            in_=bass.AP(tensor=hi1.tensor, offset=hi1.offset,
                        ap=[list(hi1.ap[0]), [2, N // 2]]))

        # ---- padded sequence ----
        t_seq = spool.tile([C, L + DC - 1], f16)
        nc.vector.memset(t_seq[:, 0:DC - 1], 0.0)
        nc.sync.dma_start(out=t_seq[:, DC - 1:], in_=seqT[:, :])

        # persistent activations; hi-group dt/u replicated twice along partitions
        t_xa = [big_pool.tile([PLO, L], f16, name="xa0"),
                big_pool.tile([PHI, L], f16, name="xa1")]
        t_zs = [big_pool.tile([PLO, L], f16, name="zs0"),
                big_pool.tile([PHI, L], f16, name="zs1")]
        t_dt = [big_pool.tile([PLO, L], f32, name="dt0"),
                big_pool.tile([PLO, L], f32, name="dt1rep")]
        t_u = [big_pool.tile([PLO, L], f16, name="u0"),
               big_pool.tile([PLO, L], f16, name="u1rep")]
        t_proj = big_pool.tile([DTR + 2 * N, L], f16, name="proj")
        t_stlo = big_pool.tile([PLO, N], f32, name="stlo")
        t_sthi = big_pool.tile([PLO, N // 2], f32, name="sthi")

        bc_dram = dram_pool.tile([2 * N, L], f16)

        PW = [PLO, PHI]

        def body(_iv=None):
            # ================= phase A =================
            for s in range(L // MCH):
                g0 = s * MCH
                for i in range(2):
                    pw = PW[i]
                    ps_x = ps_pool.tile([PLO, MCH], f32, tag="ps",
                                        name=f"psx{i}_{s}")
                    for j in range(DC):
                        nc.tensor.matmul(ps_x[:pw, :],
                                         t_wc[:, j, i * PLO:i * PLO + pw],
                                         t_seq[:, g0 + j: g0 + j + MCH],
                                         start=(j == 0), stop=(j == DC - 1))
                    xv = tmp_pool.tile([PLO, MCH], f32, tag="xv",
                                       name=f"xv{i}_{s}")
                    nc.scalar.activation(out=xv[:pw], in_=ps_x[:pw],
                                         func=Act.Identity, bias=t_cb[i])
                    sg = tmp_pool.tile([PLO, MCH], f32, tag="sg",
                                       name=f"sg{i}_{s}")
                    nc.scalar.activation(out=sg[:pw], in_=ps_x[:pw],
                                         func=Act.Exp, scale=-1.0,
                                         bias=t_cbn[i])
                    nc.gpsimd.tensor_scalar_add(sg[:pw], sg[:pw], 1.0)
                    nc.vector.reciprocal(out=sg[:pw], in_=sg[:pw])
                    nc.gpsimd.tensor_tensor(out=t_xa[i][:, g0:g0 + MCH],
                                            in0=xv[:pw], in1=sg[:pw],
                                            op=Alu.mult)

                    ps_z = ps_pool.tile([PLO, MCH], f32, tag="ps",
                                        name=f"psz{i}_{s}")
                    nc.tensor.matmul(ps_z[:pw, :],
                                     t_wz[:, i * PLO:i * PLO + pw],
                                     t_seq[:, g0 + DC - 1: g0 + DC - 1 + MCH],
                                     start=True, stop=True)
                    zg = tmp_pool.tile([PLO, MCH], f32, tag="zg",
                                       name=f"zg{i}_{s}")
                    nc.scalar.activation(out=zg[:pw], in_=ps_z[:pw],
                                         func=Act.Exp, scale=-1.0)
                    nc.gpsimd.tensor_scalar_add(zg[:pw], zg[:pw], 1.0)
                    nc.vector.reciprocal(out=zg[:pw], in_=zg[:pw])
                    nc.vector.scalar_tensor_tensor(
                        out=t_zs[i][:, g0:g0 + MCH], in0=ps_z[:pw], scalar=1.0,
                        in1=zg[:pw], op0=Alu.mult, op1=Alu.mult)

            for s in range(L // MCH):
                g0 = s * MCH
                ps_proj = ps_pool.tile([DTR + 2 * N, MCH], f32, tag="ps",
                                       name=f"psp_{s}")
                for i in range(2):
                    nc.tensor.matmul(ps_proj[:, :], t_wxT[i],
                                     t_xa[i][:, g0:g0 + MCH],
                                     start=(i == 0), stop=(i == 1))
                nc.scalar.activation(out=t_proj[:, g0:g0 + MCH], in_=ps_proj,
                                     func=Act.Copy)
                nc.gpsimd.dma_start(out=bc_dram[:, g0:g0 + MCH],
                                    in_=t_proj[DTR:, g0:g0 + MCH])

            # softplus: all Exps -> t_dt, then one add1 + in-place Ln per group
            for s in range(L // MCH):
                g0 = s * MCH
                for i in range(2):
                    pw = PW[i]
                    ps_dt = ps_pool.tile([PLO, MCH], f32, tag="ps",
                                         name=f"psdt{i}_{s}")
                    nc.tensor.matmul(ps_dt[:pw, :],
                                     t_wdtT[:, i * PLO:i * PLO + pw],
                                     t_proj[0:DTR, g0:g0 + MCH],
                                     start=True, stop=True)
                    nc.scalar.activation(out=t_dt[i][:pw, g0:g0 + MCH],
                                         in_=ps_dt[:pw], func=Act.Exp,
                                         bias=t_bdt[i])
            for i in range(2):
                pw = PW[i]
                nc.vector.tensor_scalar_add(t_dt[i][:pw], t_dt[i][:pw], 1.0)
                nc.scalar.activation(out=t_dt[i][:pw], in_=t_dt[i][:pw],
                                     func=Act.Ln)
                nc.vector.tensor_tensor(out=t_u[i][:pw], in0=t_dt[i][:pw],
                                        in1=t_xa[i][:, :], op=Alu.mult)
            # replicate hi rows [0:64] -> [64:128]
            nc.vector.tensor_copy(t_dt[1][PHI:PLO, :], t_dt[1][0:PHI, :])
            nc.vector.tensor_copy(t_u[1][PHI:PLO, :], t_u[1][0:PHI, :])

            # ================= phase B + C per section =================
            nc.vector.memset(t_stlo, 0.0)
            nc.vector.memset(t_sthi, 0.0)
            # items: ("lo", n) x16 and ("hi", j) x8 (pair 2j, 2j+1)
            items = []
            for j in range(N // 2):
                items.append((0, 2 * j))
                items.append((0, 2 * j + 1))
                items.append((1, j))
            for si in range(NSEC):
                s0 = si * SEC
                ya_ps = [ya_ps_pool.tile([PLO, SEC], f32, tag="yaps0",
                                         name=f"yaps0_{si}"),
                         ya_ps_pool.tile([PHI, SEC], f32, tag="yaps1",
                                         name=f"yaps1_{si}")]
                seen = [0, 0]
                for g, n in items:
                    first = seen[g] == 0
                    seen[g] += 1
                    last = seen[g] == (N if g == 0 else N // 2)
                    # B and C rows broadcast in one DMA: bcc[:, 0, :] = B,
                    # bcc[:, 1, :] = C (partition-broadcast from DRAM)
                    bcc = bc_pool.tile([PLO, 2, SEC], f16, tag="bcc", bufs=8,
                                       name=f"bcc_{si}_{g}_{n}")
                    if g == 0:
                        src = bc_dram[n, s0:s0 + SEC]
                        nc.sync.dma_start(
                            out=bcc, in_=bass.AP(
                                tensor=src.tensor, offset=src.offset,
                                ap=[[0, PLO], [N * L, 2]] + list(src.ap)))
                    else:
                        for half in range(2):
                            src = bc_dram[2 * n + half, s0:s0 + SEC]
                            nc.sync.dma_start(
                                out=bcc[half * PHI:(half + 1) * PHI, :, :],
                                in_=bass.AP(
                                    tensor=src.tensor, offset=src.offset,
                                    ap=[[0, PHI], [N * L, 2]] + list(src.ap)))
                    bmb = bcc[:, 0, :]
                    cmb = bcc[:, 1, :]
                    acol_t = t_aclo if g == 0 else t_achi
                    st_t = t_stlo if g == 0 else t_sthi
                    dA = n_pool.tile([PLO, SEC], f16, tag="dA",
                                     name=f"dA_{si}_{g}_{n}")
                    nc.scalar.activation(out=dA, in_=t_dt[g][:, s0:s0 + SEC],
                                         func=Act.Exp,
                                         scale=acol_t[:, n:n + 1])
                    dBx = n_pool.tile([PLO, SEC], f16, tag="dBx",
                                      name=f"dBx_{si}_{g}_{n}")
                    (nc.gpsimd if g == 1 else nc.vector).tensor_tensor(
                        out=dBx, in0=t_u[g][:, s0:s0 + SEC],
                        in1=bmb, op=Alu.mult)
                    h = n_pool.tile([PLO, SEC], f16, tag="h",
                                    name=f"h_{si}_{g}_{n}")
                    nc.vector.tensor_tensor_scan(
                        out=h, data0=dA, data1=dBx,
                        initial=st_t[:, n:n + 1],
                        op0=Alu.mult, op1=Alu.add)
                    nc.vector.tensor_copy(st_t[:, n:n + 1], h[:, SEC - 1:SEC])
                    hc = n_pool.tile([PLO, SEC], f16, tag="hc",
                                     name=f"hc_{si}_{g}_{n}")
                    (nc.gpsimd if pool_hc else nc.vector).tensor_tensor(
                        out=hc, in0=h, in1=cmb, op=Alu.mult)
                    lhs = (t_idsel[:, 0:PLO] if g == 0
                           else t_idsel[:, PLO:PLO + PHI])
                    for q in range(SEC // MCH):
                        nc.tensor.matmul(
                            ya_ps[g][:, q * MCH:(q + 1) * MCH], lhs,
                            hc[:, q * MCH:(q + 1) * MCH],
                            start=first, stop=last)

                for q in range(SEC // MCH):
                    g0 = s0 + q * MCH
                    ps_o = ps_pool.tile([C, MCH], f32, tag="ps",
                                        name=f"pso_{si}_{q}")
                    for i in range(2):
                        pw = PW[i]
                        yg = tmp_pool.tile([PLO, MCH], f32, tag="yg",
                                           name=f"yg{i}_{si}_{q}")
                        nc.vector.scalar_tensor_tensor(
                            out=yg[:pw], in0=t_xa[i][:, g0:g0 + MCH],
                            scalar=t_dpv[i],
                            in1=ya_ps[i][:pw, q * MCH:(q + 1) * MCH],
                            op0=Alu.mult, op1=Alu.add)
                        ygz = tmp_pool.tile([PLO, MCH], f16, tag="ygz",
                                            name=f"ygz{i}_{si}_{q}")
                        (nc.gpsimd if pool_c else nc.vector).tensor_tensor(
                            out=ygz[:pw], in0=yg[:pw],
                            in1=t_zs[i][:, g0:g0 + MCH], op=Alu.mult)
                        nc.tensor.matmul(ps_o[:, :], t_woT[i], ygz[:pw, :],
                                         start=(i == 0), stop=(i == 1))
                    o_sb = tmp_pool.tile([C, MCH], f32, tag="osb",
                                         name=f"osb{si}_{q}")
                    nc.scalar.activation(out=o_sb, in_=ps_o, func=Act.Copy)
                    nc.gpsimd.dma_start(out=out[:, g0:g0 + MCH], in_=o_sb)

        if repeat == 1:
            body()
        else:
            with tc.For_i(0, repeat, 1) as iv:
                body(iv)

    nc.compile()
    return nc



def _prep_core_inputs(inp, d, seqT):
    W_in = inp['W_in'][d]
    conv_w = inp['conv_w'][d]
    A = -np.exp(inp['A_log'][d])
    wc = np.einsum('pc,pj->cjp', W_in[:P, :], conv_w)       # (C, DC, P)
    idsel = np.concatenate(
        [np.eye(PLO, dtype=np.float16),
         np.vstack([np.eye(PHI, dtype=np.float16)] * 2)], axis=1)
    return {
        'seqT': np.ascontiguousarray(seqT).astype(np.float16),
        'wc': np.ascontiguousarray(wc).astype(np.float16),
        'wz': np.ascontiguousarray(W_in[P:, :].T).astype(np.float16),
        'wxT': np.ascontiguousarray(inp['W_x'][d].T).astype(np.float16),
        'wdtT': np.ascontiguousarray(inp['W_dt'][d].T).astype(np.float16),
        'woT': np.ascontiguousarray(inp['W_out'][d].T).astype(np.float16),
        'idsel': idsel,
        'bdt': np.ascontiguousarray(inp['b_dt'][d][:, None], np.float32),
        'cb': np.ascontiguousarray(inp['conv_b'][d][:, None], np.float32),
        'cbn': np.ascontiguousarray(-inp['conv_b'][d][:, None], np.float32),
        'dpv': np.ascontiguousarray(inp['Dp'][d][:, None], np.float32),
        'acol': np.ascontiguousarray(A, np.float32),
    }


def kernel(x1, x2, W_in, conv_w, conv_b, W_x, W_dt, b_dt, A_log, Dp, W_out):
    from concourse.bass_utils import run_bass_kernel_spmd

    inp = dict(x1=np.asarray(x1), x2=np.asarray(x2), W_in=np.asarray(W_in),
               conv_w=np.asarray(conv_w), conv_b=np.asarray(conv_b),
               W_x=np.asarray(W_x), W_dt=np.asarray(W_dt),
               b_dt=np.asarray(b_dt), A_log=np.asarray(A_log),
               Dp=np.asarray(Dp), W_out=np.asarray(W_out))
    B = inp['x1'].shape[0]

    if 'nc' not in _CACHED:
        _CACHED['nc'] = _build_program()
    nc = _CACHED['nc']

    in_maps = []
    metas = []
    for d in range(4):
        for b in range(B):
            x = inp['x1'][b]
            if d < 2:
                seq = x.reshape(C, L)
            else:
                seq = np.ascontiguousarray(x.transpose(0, 2, 1)).reshape(C, L)
            if d in (1, 3):
                seq = seq[:, ::-1]
            in_maps.append(_prep_core_inputs(inp, d, seq))
            metas.append((d, b))

    res = run_bass_kernel_spmd(nc, in_maps, core_ids=list(range(len(in_maps))))

    outs = np.zeros((B, C, HH, WW), np.float32)
    for (d, b), r in zip(metas, res.results):
        y = r['out']                      # (C, L)
        if d in (1, 3):
            y = y[:, ::-1]
        if d < 2:
            y = y.reshape(C, HH, WW)
        else:
            y = y.reshape(C, WW, HH).transpose(0, 2, 1)
        outs[b] += y
    outs += inp['x2']
    return outs



# revision 6
# speedup vs baseline: 32.6801x; 32.6801x over previous
"""Trainium2 Bass kernel for the 4-directional Mamba (SS2D / VMamba-style)
block from the OSS reference.

Sharding: the 8 independent (direction x batch) sequences map one-per-core
(SPMD: one NEFF, 8 cores, per-core inputs). Backward directions are handled by
host-side flips of the input/output sequences; the final sum of the four
directional outputs plus the residual x2 happens at gather time on host.

Per-core kernel (C=96, L=4096, P=192, N=16, dtr=6), v2:
  - causal depthwise conv folded into the input projection as 4 shifted
    tap-matmuls accumulating in PSUM (PE, fp16 operands)
  - silu / softplus computed directly by the ACT engine table functions
  - W_x output columns permuted host-side to [dtr | B even n | B odd n |
    C even | C odd] so the B/C planes written to DRAM are contiguous both
    for the lo-group (all 16 n) and the hi-group (even n -> top half,
    odd n -> bottom half) broadcast loads
  - B/C rows round-trip through DRAM in a section-major plane layout so the
    partition-broadcast loads coalesce into 2 lo-group + 2 hi-half DMAs per
    section with fat (16KB) descriptors — broadcast DMA cost on this HW is
    ~24ns/descriptor, so descriptor count is everything
  - the selective scan runs on the native DVE tensor_tensor_scan (f16, the
    fastest measured variant given the surrounding f16 2x-mode multiplies);
    P=192 states packed as a full 128-row group plus a pair-packed 64-row
    group
  - state hand-off between sections via 1-column ACT copies into st strips
  - dBx / hc multiplies split between DVE (f16 2x mode, ~0.8us/tile) and
    Pool (~2.4us/tile) to balance engine busy time; sum over n runs on the
    PE as identity-selection matmuls accumulating in PSUM
"""

import numpy as np

C = 96
L = 4096
P = 192
PLO = 128
PHI = 64
N = 16
DTR = 6
DC = 4
HH = 64
WW = 64
MCH = 512
NSEC = 4
SEC = L // NSEC

# lo-group item order: evens then odds (matches the permuted W_x columns)
NPERM = [2 * j for j in range(N // 2)] + [2 * j + 1 for j in range(N // 2)]

_CACHED = {}


def _build_program(repeat=1, n_dve_hc=4, n_dve_dbx=16):
    """n_dve_hc / n_dve_dbx: how many of the 16 lo hc / dBx mults per section
    run on DVE (the rest run on Pool) — engine balance knobs."""
    from contextlib import ExitStack

    import concourse.bacc as bacc
    import concourse.bass as bass
    import concourse.tile as tile
    from concourse import mybir

    f32 = mybir.dt.float32
    f16 = mybir.dt.float16
    Alu = mybir.AluOpType
    Act = mybir.ActivationFunctionType

    nc = bacc.Bacc()

    seqT = nc.dram_tensor("seqT", [C, L], f16, kind="ExternalInput")
    wc = nc.dram_tensor("wc", [C, DC, P], f16, kind="ExternalInput")
    wz = nc.dram_tensor("wz", [C, P], f16, kind="ExternalInput")
    wxT = nc.dram_tensor("wxT", [P, DTR + 2 * N], f16, kind="ExternalInput")
    wdtT = nc.dram_tensor("wdtT", [DTR, P], f16, kind="ExternalInput")
    woT = nc.dram_tensor("woT", [P, C], f16, kind="ExternalInput")
    idsel = nc.dram_tensor("idsel", [PLO, PLO + PHI], f16, kind="ExternalInput")
    bdt = nc.dram_tensor("bdt", [P, 1], f32, kind="ExternalInput")
    cb = nc.dram_tensor("cb", [P, 1], f32, kind="ExternalInput")
    dpv = nc.dram_tensor("dpv", [P, 1], f32, kind="ExternalInput")
    acol = nc.dram_tensor("acol", [P, N], f32, kind="ExternalInput")
    out = nc.dram_tensor("out", [C, L], f32, kind="ExternalOutput")

    with tile.TileContext(nc) as tc, ExitStack() as ctx:
        wpool = ctx.enter_context(tc.tile_pool(name="weights", bufs=1))
        spool = ctx.enter_context(tc.tile_pool(name="seq", bufs=1))
        big_pool = ctx.enter_context(tc.tile_pool(name="big", bufs=1))
        tmp_pool = ctx.enter_context(tc.tile_pool(name="tmp", bufs=2))
        n_pool = ctx.enter_context(tc.tile_pool(name="nl", bufs=3))
        bc_pool = ctx.enter_context(tc.tile_pool(name="bc", bufs=1))
        ps_pool = ctx.enter_context(tc.tile_pool(name="ps", bufs=3, space="PSUM"))
        ya_ps_pool = ctx.enter_context(
            tc.tile_pool(name="yps", bufs=1, space="PSUM"))
        dram_pool = ctx.enter_context(tc.tile_pool(name="dr", bufs=1, space="DRAM"))

        # ---- weights (lo = p 0:128, hi = p 128:192) ----
        t_wc = wpool.tile([C, DC, P], f16)
        t_wz = wpool.tile([C, P], f16)
        t_wxT = [wpool.tile([PLO, DTR + 2 * N], f16, name="wxT0"),
                 wpool.tile([PHI, DTR + 2 * N], f16, name="wxT1")]
        t_wdtT = wpool.tile([DTR, P], f16)
        t_woT = [wpool.tile([PLO, C], f16, name="woT0"),
                 wpool.tile([PHI, C], f16, name="woT1")]
        t_idsel = wpool.tile([PLO, PLO + PHI], f16)
        t_bdt = [wpool.tile([PLO, 1], f32, name="bdt0"),
                 wpool.tile([PHI, 1], f32, name="bdt1")]
        t_cb = [wpool.tile([PLO, 1], f32, name="cb0"),
                wpool.tile([PHI, 1], f32, name="cb1")]
        t_dpv = [wpool.tile([PLO, 1], f32, name="dpv0"),
                 wpool.tile([PHI, 1], f32, name="dpv1")]
        t_aclo = wpool.tile([PLO, N], f32)
        t_achi = wpool.tile([PLO, N // 2], f32)

        nc.sync.dma_start(out=t_wc, in_=wc[...])
        nc.sync.dma_start(out=t_wz, in_=wz[...])
        nc.sync.dma_start(out=t_wdtT, in_=wdtT[...])
        nc.sync.dma_start(out=t_idsel, in_=idsel[...])
        for i, (a, b) in enumerate([(0, PLO), (PLO, P)]):
            nc.sync.dma_start(out=t_wxT[i], in_=wxT[a:b, :])
            nc.sync.dma_start(out=t_woT[i], in_=woT[a:b, :])
            nc.sync.dma_start(out=t_bdt[i], in_=bdt[a:b, :])
            nc.sync.dma_start(out=t_cb[i], in_=cb[a:b, :])
            nc.sync.dma_start(out=t_dpv[i], in_=dpv[a:b, :])
        # lo A columns in NPERM order (matches lo item order)
        lo = acol[0:PLO, :]
        nc.sync.dma_start(
            out=t_aclo[:, 0:N // 2],
            in_=bass.AP(tensor=lo.tensor, offset=lo.offset,
                        ap=[list(lo.ap[0]), [2, N // 2]]))
        lo1 = acol[0:PLO, 1:]
        nc.sync.dma_start(
            out=t_aclo[:, N // 2:N],
            in_=bass.AP(tensor=lo1.tensor, offset=lo1.offset,
                        ap=[list(lo1.ap[0]), [2, N // 2]]))
        # hi pair-packed A columns: col j = [A[128:192, 2j]; A[128:192, 2j+1]]
        hi = acol[PLO:P, :]
        nc.sync.dma_start(
            out=t_achi[0:PHI, :],
            in_=bass.AP(tensor=hi.tensor, offset=hi.offset,
                        ap=[list(hi.ap[0]), [2, N // 2]]))
        hi1 = acol[PLO:P, 1:]
        nc.sync.dma_start(
            out=t_achi[PHI:PLO, :],
            in_=bass.AP(tensor=hi1.tensor, offset=hi1.offset,
                        ap=[list(hi1.ap[0]), [2, N // 2]]))

        # ---- padded sequence ----
        t_seq = spool.tile([C, L + DC - 1], f16)
        nc.vector.memset(t_seq[:, 0:DC - 1], 0.0)
        nc.sync.dma_start(out=t_seq[:, DC - 1:], in_=seqT[:, :])

        # persistent activations; hi-group dt/u replicated twice on partitions
        t_xa = [big_pool.tile([PLO, L], f16, name="xa0"),
                big_pool.tile([PHI, L], f16, name="xa1")]
        t_zs = [big_pool.tile([PLO, L], f16, name="zs0"),
                big_pool.tile([PHI, L], f16, name="zs1")]
        t_dt = [big_pool.tile([PLO, L], f16, name="dt0"),
                big_pool.tile([PLO, L], f16, name="dt1rep")]
        t_u = [big_pool.tile([PLO, L], f16, name="u0"),
               big_pool.tile([PLO, L], f16, name="u1rep")]
        t_proj = big_pool.tile([DTR + 2 * N, L], f16, name="proj")
        t_stlo = big_pool.tile([PLO, N], f32, name="stlo")
        t_sthi = big_pool.tile([PLO, N // 2], f32, name="sthi")

        # DRAM staging: bc_lo[s, r, k, :] with r=0 -> B, r=1 -> C and k in
        # NPERM order (k<8: even n, k>=8: odd n), section-major.
        bc_lo = dram_pool.tile([NSEC, 2, N, SEC], f16)

        PW = [PLO, PHI]

        def body(_iv=None):
            # ================= phase A =================
            for s in range(L // MCH):
                g0 = s * MCH
                for i in range(2):
                    pw = PW[i]
                    ps_x = ps_pool.tile([PLO, MCH], f32, tag="ps",
                                        name=f"psx{i}_{s}")
                    for j in range(DC):
                        nc.tensor.matmul(ps_x[:pw, :],
                                         t_wc[:, j, i * PLO:i * PLO + pw],
                                         t_seq[:, g0 + j: g0 + j + MCH],
                                         start=(j == 0), stop=(j == DC - 1))
                    nc.scalar.activation(out=t_xa[i][:, g0:g0 + MCH],
                                         in_=ps_x[:pw], func=Act.Silu,
                                         bias=t_cb[i])

                    ps_z = ps_pool.tile([PLO, MCH], f32, tag="ps",
                                        name=f"psz{i}_{s}")
                    nc.tensor.matmul(ps_z[:pw, :],
                                     t_wz[:, i * PLO:i * PLO + pw],
                                     t_seq[:, g0 + DC - 1: g0 + DC - 1 + MCH],
                                     start=True, stop=True)
                    nc.scalar.activation(out=t_zs[i][:, g0:g0 + MCH],
                                         in_=ps_z[:pw], func=Act.Silu)

            for s in range(L // MCH):
                g0 = s * MCH
                ps_proj = ps_pool.tile([DTR + 2 * N, MCH], f32, tag="ps",
                                       name=f"psp_{s}")
                for i in range(2):
                    nc.tensor.matmul(ps_proj[:, :], t_wxT[i],
                                     t_xa[i][:, g0:g0 + MCH],
                                     start=(i == 0), stop=(i == 1))
                nc.scalar.activation(out=t_proj[:, g0:g0 + MCH], in_=ps_proj,
                                     func=Act.Copy)

            # dtm = -softplus(raw) built as ln(sigmoid(-raw)); staged through
            # t_u so the sigmoid and ln passes each load their table once
            for s in range(L // MCH):
                g0 = s * MCH
                for i in range(2):
                    pw = PW[i]
                    ps_dt = ps_pool.tile([PLO, MCH], f32, tag="ps",
                                         name=f"psdt{i}_{s}")
                    nc.tensor.matmul(ps_dt[:pw, :],
                                     t_wdtT[:, i * PLO:i * PLO + pw],
                                     t_proj[0:DTR, g0:g0 + MCH],
                                     start=True, stop=True)
                    nc.scalar.activation(out=t_u[i][:pw, g0:g0 + MCH],
                                         in_=ps_dt[:pw], func=Act.Sigmoid,
                                         scale=-1.0, bias=t_bdt[i])
            for i in range(2):
                pw = PW[i]
                nc.scalar.activation(out=t_dt[i][:pw], in_=t_u[i][:pw],
                                     func=Act.Ln)
                nc.vector.tensor_tensor(out=t_u[i][:pw], in0=t_dt[i][:pw],
                                        in1=t_xa[i][:, :], op=Alu.mult)
            # replicate hi rows [0:64] -> [64:128]
            nc.vector.tensor_copy(t_dt[1][PHI:PLO, :], t_dt[1][0:PHI, :])
            nc.vector.tensor_copy(t_u[1][PHI:PLO, :], t_u[1][0:PHI, :])

            # ---- write B/C planes to DRAM (section-major, coalescible) ----
            for r in range(2):
                src = t_proj[DTR + r * N:DTR + (r + 1) * N, :]
                nc.gpsimd.dma_start(
                    out=bass.AP(tensor=bc_lo.tensor,
                                offset=bc_lo.offset + r * N * SEC,
                                ap=[[SEC, N], [2 * N * SEC, NSEC], [1, SEC]]),
                    in_=bass.AP(tensor=src.tensor, offset=src.offset,
                                ap=[list(src.ap[0]), [SEC, NSEC], [1, SEC]]))

            # ================= phase B + C per section =================
            nc.vector.memset(t_stlo, 0.0)
            nc.vector.memset(t_sthi, 0.0)
            dma_engs = [nc.sync, nc.scalar]
            for si in range(NSEC):
                s0 = si * SEC
                sec_off = bc_lo.offset + si * 2 * N * SEC
                # lo broadcasts: 2 groups of 8 items, (128, 2, 8, SEC) each
                t_blo = [bc_pool.tile([PLO, 2, 8, SEC], f16, tag=f"blo{g}",
                                      name=f"blo{g}_{si}") for g in range(2)]
                for g in range(2):
                    dma_engs[g].dma_start(
                        out=t_blo[g],
                        in_=bass.AP(tensor=bc_lo.tensor,
                                    offset=sec_off + 8 * g * SEC,
                                    ap=[[0, PLO], [N * SEC, 2], [SEC, 8],
                                        [1, SEC]]))
                # hi broadcast: even n rows (k 0:8) -> top half, odd n rows
                # (k 8:16) -> bottom half
                t_bhi = bc_pool.tile([PLO, 2, N // 2, SEC], f16, tag="bhi",
                                     name=f"bhi_{si}")
                for par in range(2):
                    dma_engs[par].dma_start(
                        out=t_bhi[par * PHI:(par + 1) * PHI, :, :, :],
                        in_=bass.AP(tensor=bc_lo.tensor,
                                    offset=sec_off + 8 * par * SEC,
                                    ap=[[0, PHI], [N * SEC, 2], [SEC, 8],
                                        [1, SEC]]))

                ya_ps = [ya_ps_pool.tile([PLO, SEC], f32, tag="yaps0",
                                         name=f"yaps0_{si}"),
                         ya_ps_pool.tile([PHI, SEC], f32, tag="yaps1",
                                         name=f"yaps1_{si}")]
                # items: lo k = 0..15 (NPERM order) interleaved with hi j
                items = []
                for j in range(N // 2):
                    items.append((0, j))            # lo even block
                    items.append((0, N // 2 + j))   # lo odd block
                    items.append((1, j))            # hi pair j
                seen = [0, 0]
                ndve = [0, 0]
                for g, k in items:
                    first = seen[g] == 0
                    seen[g] += 1
                    last = seen[g] == (N if g == 0 else N // 2)
                    if g == 0:
                        bmb = t_blo[k // 8][:, 0, k % 8, :]
                        cmb = t_blo[k // 8][:, 1, k % 8, :]
                    else:
                        bmb = t_bhi[:, 0, k, :]
                        cmb = t_bhi[:, 1, k, :]
                    acol_t = t_aclo if g == 0 else t_achi
                    st_t = t_stlo if g == 0 else t_sthi
                    dA = n_pool.tile([PLO, SEC], f16, tag="dA", bufs=3,
                                     name=f"dA_{si}_{g}_{k}")
                    nc.scalar.activation(out=dA, in_=t_dt[g][:, s0:s0 + SEC],
                                         func=Act.Exp,
                                         scale=acol_t[:, k:k + 1])
                    dBx = n_pool.tile([PLO, SEC], f16, tag="dBx", bufs=2,
                                      name=f"dBx_{si}_{g}_{k}")
                    use_dve_dbx = g == 0 and ndve[0] < n_dve_dbx
                    if use_dve_dbx:
                        ndve[0] += 1
                    (nc.vector if use_dve_dbx else nc.gpsimd).tensor_tensor(
                        out=dBx, in0=t_u[g][:, s0:s0 + SEC],
                        in1=bmb, op=Alu.mult)
                    h = n_pool.tile([PLO, SEC], f16, tag="h", bufs=3,
                                    name=f"h_{si}_{g}_{k}")
                    nc.vector.tensor_tensor_scan(
                        out=h, data0=dA, data1=dBx,
                        initial=st_t[:, k:k + 1],
                        op0=Alu.mult, op1=Alu.add)
                    if si < NSEC - 1:
                        nc.scalar.activation(out=st_t[:, k:k + 1],
                                             in_=h[:, SEC - 1:SEC],
                                             func=Act.Copy)
                    hc = n_pool.tile([PLO, SEC], f16, tag="hc", bufs=2,
                                     name=f"hc_{si}_{g}_{k}")
                    use_dve_hc = (g == 0 and ndve[1] < n_dve_hc)
                    if use_dve_hc:
                        ndve[1] += 1
                    (nc.vector if use_dve_hc else nc.gpsimd).tensor_tensor(
                        out=hc, in0=h, in1=cmb, op=Alu.mult)
                    lhs = (t_idsel[:, 0:PLO] if g == 0
                           else t_idsel[:, PLO:PLO + PHI])
                    for q in range(SEC // MCH):
                        nc.tensor.matmul(
                            ya_ps[g][:, q * MCH:(q + 1) * MCH], lhs,
                            hc[:, q * MCH:(q + 1) * MCH],
                            start=first, stop=last)

                for q in range(SEC // MCH):
                    g0 = s0 + q * MCH
                    ps_o = ps_pool.tile([C, MCH], f32, tag="ps",
                                        name=f"pso_{si}_{q}")
                    for i in range(2):
                        pw = PW[i]
                        yg = tmp_pool.tile([PLO, MCH], f16, tag="yg",
                                           name=f"yg{i}_{si}_{q}")
                        nc.vector.scalar_tensor_tensor(
                            out=yg[:pw], in0=t_xa[i][:, g0:g0 + MCH],
                            scalar=t_dpv[i],
                            in1=ya_ps[i][:pw, q * MCH:(q + 1) * MCH],
                            op0=Alu.mult, op1=Alu.subtract)
                        ygz = tmp_pool.tile([PLO, MCH], f16, tag="ygz",
                                            name=f"ygz{i}_{si}_{q}")
                        nc.gpsimd.tensor_tensor(
                            out=ygz[:pw], in0=yg[:pw],
                            in1=t_zs[i][:, g0:g0 + MCH], op=Alu.mult)
                        nc.tensor.matmul(ps_o[:, :], t_woT[i], ygz[:pw, :],
                                         start=(i == 0), stop=(i == 1))
                    o_sb = tmp_pool.tile([C, MCH], f32, tag="osb",
                                         name=f"osb{si}_{q}")
                    nc.scalar.activation(out=o_sb, in_=ps_o, func=Act.Copy)
                    nc.gpsimd.dma_start(out=out[:, g0:g0 + MCH], in_=o_sb)

        if repeat == 1:
            body()
        else:
            with tc.For_i(0, repeat, 1) as iv:
                body(iv)

    nc.compile()
    return nc


def _prep_core_inputs(inp, d, seqT):
    W_in = inp['W_in'][d]
    conv_w = inp['conv_w'][d]
    A = np.exp(inp['A_log'][d])   # positive: dA = exp(dtm * (-A_true))
    wc = np.einsum('pc,pj->cjp', W_in[:P, :], conv_w)       # (C, DC, P)
    idsel = np.concatenate(
        [np.eye(PLO, dtype=np.float16),
         np.vstack([np.eye(PHI, dtype=np.float16)] * 2)], axis=1)
    # permute W_x columns (proj rows): [dtr | B NPERM | C NPERM]
    cols = (list(range(DTR)) + [DTR + k for k in NPERM]
            + [DTR + N + k for k in NPERM])
    wx_perm = inp['W_x'][d][cols, :]
    return {
        'seqT': np.ascontiguousarray(seqT).astype(np.float16),
        'wc': np.ascontiguousarray(wc).astype(np.float16),
        'wz': np.ascontiguousarray(W_in[P:, :].T).astype(np.float16),
        'wxT': np.ascontiguousarray(wx_perm.T).astype(np.float16),
        'wdtT': np.ascontiguousarray(inp['W_dt'][d].T).astype(np.float16),
        'woT': np.ascontiguousarray(inp['W_out'][d].T).astype(np.float16),
        'idsel': idsel,
        'bdt': np.ascontiguousarray(-inp['b_dt'][d][:, None], np.float32),
        'cb': np.ascontiguousarray(inp['conv_b'][d][:, None], np.float32),
        'dpv': np.ascontiguousarray(inp['Dp'][d][:, None], np.float32),
        'acol': np.ascontiguousarray(A, np.float32),
    }


def kernel(x1, x2, W_in, conv_w, conv_b, W_x, W_dt, b_dt, A_log, Dp, W_out):
    from concourse.bass_utils import run_bass_kernel_spmd

    inp = dict(x1=np.asarray(x1), x2=np.asarray(x2), W_in=np.asarray(W_in),
               conv_w=np.asarray(conv_w), conv_b=np.asarray(conv_b),
               W_x=np.asarray(W_x), W_dt=np.asarray(W_dt),
               b_dt=np.asarray(b_dt), A_log=np.asarray(A_log),
               Dp=np.asarray(Dp), W_out=np.asarray(W_out))
    B = inp['x1'].shape[0]

    if 'nc' not in _CACHED:
        _CACHED['nc'] = _build_program()
    nc = _CACHED['nc']

    in_maps = []
    metas = []
    for d in range(4):
        for b in range(B):
            x = inp['x1'][b]
            if d < 2:
                seq = x.reshape(C, L)
            else:
                seq = np.ascontiguousarray(x.transpose(0, 2, 1)).reshape(C, L)
            if d in (1, 3):
                seq = seq[:, ::-1]
            in_maps.append(_prep_core_inputs(inp, d, seq))
            metas.append((d, b))

    res = run_bass_kernel_spmd(nc, in_maps, core_ids=list(range(len(in_maps))))

    outs = np.zeros((B, C, HH, WW), np.float32)
    for (d, b), r in zip(metas, res.results):
        y = r['out']                      # (C, L)
        if d in (1, 3):
            y = y[:, ::-1]
        if d < 2:
            y = y.reshape(C, HH, WW)
        else:
            y = y.reshape(C, WW, HH).transpose(0, 2, 1)
        outs[b] += y
    outs += inp['x2']
    return outs


# revision 9
# speedup vs baseline: 46.0362x; 1.4087x over previous
"""Trainium2 Bass kernel for the 4-directional Mamba (SS2D / VMamba-style)
block from the OSS reference.

Sharding: the 8 independent (direction x batch) sequences map one-per-core
(SPMD: one NEFF, 8 cores, per-core inputs). Backward directions are handled by
host-side flips of the input/output sequences; the final sum of the four
directional outputs plus the residual x2 happens at gather time on host.

Per-core kernel (C=96, L=4096, P=192, N=16, dtr=6), v2:
  - causal depthwise conv folded into the input projection as 4 shifted
    tap-matmuls accumulating in PSUM (PE, fp16 operands)
  - silu / softplus computed directly by the ACT engine table functions
  - W_x output columns permuted host-side to [dtr | B even n | B odd n |
    C even | C odd] so the B/C planes written to DRAM are contiguous both
    for the lo-group (all 16 n) and the hi-group (even n -> top half,
    odd n -> bottom half) broadcast loads
  - B/C rows round-trip through DRAM in a section-major plane layout so the
    partition-broadcast loads coalesce into 2 lo-group + 2 hi-half DMAs per
    section with fat (16KB) descriptors — broadcast DMA cost on this HW is
    ~24ns/descriptor, so descriptor count is everything
  - the selective scan runs on the native DVE tensor_tensor_scan (f16, the
    fastest measured variant given the surrounding f16 2x-mode multiplies);
    P=192 states packed as a full 128-row group plus a pair-packed 64-row
    group
  - state hand-off between sections via 1-column ACT copies into st strips
  - dBx / hc multiplies split between DVE (f16 2x mode, ~0.8us/tile) and
    Pool (~2.4us/tile) to balance engine busy time; sum over n runs on the
    PE as identity-selection matmuls accumulating in PSUM
"""

import numpy as np

C = 96
L = 4096
P = 192
PLO = 128
PHI = 64
N = 16
DTR = 6
DC = 4
HH = 64
WW = 64
MCH = 512
NSEC = 4
SEC = L // NSEC

# lo-group item order: evens then odds (matches the permuted W_x columns)
NPERM = [2 * j for j in range(N // 2)] + [2 * j + 1 for j in range(N // 2)]

_CACHED = {}


def _build_program(repeat=1, n_dve_hc=4, n_dve_dbx=16):
    """n_dve_hc / n_dve_dbx: how many of the 16 lo hc / dBx mults per section
    run on DVE (the rest run on Pool) — engine balance knobs."""
    from contextlib import ExitStack

    import concourse.bacc as bacc
    import concourse.bass as bass
    import concourse.tile as tile
    from concourse import mybir

    f32 = mybir.dt.float32
    f16 = mybir.dt.float16
    Alu = mybir.AluOpType
    Act = mybir.ActivationFunctionType

    nc = bacc.Bacc()

    seqT = nc.dram_tensor("seqT", [C, L], f16, kind="ExternalInput")
    wc = nc.dram_tensor("wc", [C, DC, P], f16, kind="ExternalInput")
    wz = nc.dram_tensor("wz", [C, P], f16, kind="ExternalInput")
    wxT = nc.dram_tensor("wxT", [P, DTR + 2 * N], f16, kind="ExternalInput")
    wdtT = nc.dram_tensor("wdtT", [DTR, P], f16, kind="ExternalInput")
    woT = nc.dram_tensor("woT", [P, C], f16, kind="ExternalInput")
    idsel = nc.dram_tensor("idsel", [PLO, PLO + PHI], f16, kind="ExternalInput")
    bdt = nc.dram_tensor("bdt", [P, 1], f32, kind="ExternalInput")
    cb = nc.dram_tensor("cb", [P, 1], f32, kind="ExternalInput")
    dpv = nc.dram_tensor("dpv", [P, 1], f32, kind="ExternalInput")
    acol = nc.dram_tensor("acol", [P, N], f32, kind="ExternalInput")
    out = nc.dram_tensor("out", [C, L], f32, kind="ExternalOutput")

    with tile.TileContext(nc) as tc, ExitStack() as ctx:
        wpool = ctx.enter_context(tc.tile_pool(name="weights", bufs=1))
        spool = ctx.enter_context(tc.tile_pool(name="seq", bufs=1))
        big_pool = ctx.enter_context(tc.tile_pool(name="big", bufs=1))
        tmp_pool = ctx.enter_context(tc.tile_pool(name="tmp", bufs=2))
        n_pool = ctx.enter_context(tc.tile_pool(name="nl", bufs=3))
        bc_pool = ctx.enter_context(tc.tile_pool(name="bc", bufs=1))
        ps_pool = ctx.enter_context(tc.tile_pool(name="ps", bufs=3, space="PSUM"))
        ya_ps_pool = ctx.enter_context(
            tc.tile_pool(name="yps", bufs=1, space="PSUM"))
        dram_pool = ctx.enter_context(tc.tile_pool(name="dr", bufs=1, space="DRAM"))

        # ---- weights (lo = p 0:128, hi = p 128:192) ----
        t_wc = wpool.tile([C, DC, P], f16)
        t_wz = wpool.tile([C, P], f16)
        t_wxT = [wpool.tile([PLO, DTR + 2 * N], f16, name="wxT0"),
                 wpool.tile([PHI, DTR + 2 * N], f16, name="wxT1")]
        t_wdtT = wpool.tile([DTR, P], f16)
        t_woT = [wpool.tile([PLO, C], f16, name="woT0"),
                 wpool.tile([PHI, C], f16, name="woT1")]
        t_idsel = wpool.tile([PLO, PLO + PHI], f16)
        t_bdt = [wpool.tile([PLO, 1], f32, name="bdt0"),
                 wpool.tile([PHI, 1], f32, name="bdt1")]
        t_cb = [wpool.tile([PLO, 1], f32, name="cb0"),
                wpool.tile([PHI, 1], f32, name="cb1")]
        t_dpv = [wpool.tile([PLO, 1], f32, name="dpv0"),
                 wpool.tile([PHI, 1], f32, name="dpv1")]
        t_aclo = wpool.tile([PLO, N], f32)
        t_achi = wpool.tile([PLO, N // 2], f32)

        nc.sync.dma_start(out=t_wc, in_=wc[...])
        nc.sync.dma_start(out=t_wz, in_=wz[...])
        nc.sync.dma_start(out=t_wdtT, in_=wdtT[...])
        nc.sync.dma_start(out=t_idsel, in_=idsel[...])
        for i, (a, b) in enumerate([(0, PLO), (PLO, P)]):
            nc.sync.dma_start(out=t_wxT[i], in_=wxT[a:b, :])
            nc.sync.dma_start(out=t_woT[i], in_=woT[a:b, :])
            nc.sync.dma_start(out=t_bdt[i], in_=bdt[a:b, :])
            nc.sync.dma_start(out=t_cb[i], in_=cb[a:b, :])
            nc.sync.dma_start(out=t_dpv[i], in_=dpv[a:b, :])
        # lo A columns in NPERM order (matches lo item order)
        lo = acol[0:PLO, :]
        nc.sync.dma_start(
            out=t_aclo[:, 0:N // 2],
            in_=bass.AP(tensor=lo.tensor, offset=lo.offset,
                        ap=[list(lo.ap[0]), [2, N // 2]]))
        lo1 = acol[0:PLO, 1:]
        nc.sync.dma_start(
            out=t_aclo[:, N // 2:N],
            in_=bass.AP(tensor=lo1.tensor, offset=lo1.offset,
                        ap=[list(lo1.ap[0]), [2, N // 2]]))
        # hi pair-packed A columns: col j = [A[128:192, 2j]; A[128:192, 2j+1]]
        hi = acol[PLO:P, :]
        nc.sync.dma_start(
            out=t_achi[0:PHI, :],
            in_=bass.AP(tensor=hi.tensor, offset=hi.offset,
                        ap=[list(hi.ap[0]), [2, N // 2]]))
        hi1 = acol[PLO:P, 1:]
        nc.sync.dma_start(
            out=t_achi[PHI:PLO, :],
            in_=bass.AP(tensor=hi1.tensor, offset=hi1.offset,
                        ap=[list(hi1.ap[0]), [2, N // 2]]))

        # ---- padded sequence ----
        t_seq = spool.tile([C, L + DC - 1], f16)
        nc.vector.memset(t_seq[:, 0:DC - 1], 0.0)
        nc.sync.dma_start(out=t_seq[:, DC - 1:], in_=seqT[:, :])

        # persistent activations; hi-group dt/u replicated twice on partitions
        t_xa = [big_pool.tile([PLO, L], f16, name="xa0"),
                big_pool.tile([PHI, L], f16, name="xa1")]
        t_zs = [big_pool.tile([PLO, L], f16, name="zs0"),
                big_pool.tile([PHI, L], f16, name="zs1")]
        t_dt = [big_pool.tile([PLO, L], f16, name="dt0"),
                big_pool.tile([PLO, L], f16, name="dt1rep")]
        t_u = [big_pool.tile([PLO, L], f16, name="u0"),
               big_pool.tile([PLO, L], f16, name="u1rep")]
        t_proj = big_pool.tile([DTR + 2 * N, L], f16, name="proj")
        # per-item state columns (separate tiles so items don't serialize)
        t_st = [[big_pool.tile([PLO, 1], f32, name=f"stl{k}")
                 for k in range(N)],
                [big_pool.tile([PLO, 1], f32, name=f"sth{k}")
                 for k in range(N // 2)]]

        # DRAM staging: bc_lo[s, r, k, :] with r=0 -> B, r=1 -> C and k in
        # NPERM order (k<8: even n, k>=8: odd n), section-major.
        bc_lo = dram_pool.tile([NSEC, 2, N, SEC], f16)

        PW = [PLO, PHI]

        def body(_iv=None):
            # ================= phase A =================
            for s in range(L // MCH):
                g0 = s * MCH
                for i in range(2):
                    pw = PW[i]
                    ps_x = ps_pool.tile([PLO, MCH], f32, tag="ps",
                                        name=f"psx{i}_{s}")
                    for j in range(DC):
                        nc.tensor.matmul(ps_x[:pw, :],
                                         t_wc[:, j, i * PLO:i * PLO + pw],
                                         t_seq[:, g0 + j: g0 + j + MCH],
                                         start=(j == 0), stop=(j == DC - 1))
                    nc.scalar.activation(out=t_xa[i][:, g0:g0 + MCH],
                                         in_=ps_x[:pw], func=Act.Silu,
                                         bias=t_cb[i])

                    ps_z = ps_pool.tile([PLO, MCH], f32, tag="ps",
                                        name=f"psz{i}_{s}")
                    nc.tensor.matmul(ps_z[:pw, :],
                                     t_wz[:, i * PLO:i * PLO + pw],
                                     t_seq[:, g0 + DC - 1: g0 + DC - 1 + MCH],
                                     start=True, stop=True)
                    nc.scalar.activation(out=t_zs[i][:, g0:g0 + MCH],
                                         in_=ps_z[:pw], func=Act.Silu)

            for s in range(L // MCH):
                g0 = s * MCH
                ps_proj = ps_pool.tile([DTR + 2 * N, MCH], f32, tag="ps",
                                       name=f"psp_{s}")
                for i in range(2):
                    nc.tensor.matmul(ps_proj[:, :], t_wxT[i],
                                     t_xa[i][:, g0:g0 + MCH],
                                     start=(i == 0), stop=(i == 1))
                nc.scalar.activation(out=t_proj[:, g0:g0 + MCH], in_=ps_proj,
                                     func=Act.Copy)

            # dtm = -softplus(raw) built as ln(sigmoid(-raw)); staged through
            # t_u so the sigmoid and ln passes each load their table once
            for s in range(L // MCH):
                g0 = s * MCH
                for i in range(2):
                    pw = PW[i]
                    ps_dt = ps_pool.tile([PLO, MCH], f32, tag="ps",
                                         name=f"psdt{i}_{s}")
                    nc.tensor.matmul(ps_dt[:pw, :],
                                     t_wdtT[:, i * PLO:i * PLO + pw],
                                     t_proj[0:DTR, g0:g0 + MCH],
                                     start=True, stop=True)
                    nc.scalar.activation(out=t_u[i][:pw, g0:g0 + MCH],
                                         in_=ps_dt[:pw], func=Act.Sigmoid,
                                         scale=-1.0, bias=t_bdt[i])
            for i in range(2):
                pw = PW[i]
                nc.scalar.activation(out=t_dt[i][:pw], in_=t_u[i][:pw],
                                     func=Act.Ln)
                nc.vector.tensor_tensor(out=t_u[i][:pw], in0=t_dt[i][:pw],
                                        in1=t_xa[i][:, :], op=Alu.mult)
            # replicate hi rows [0:64] -> [64:128]
            nc.vector.tensor_copy(t_dt[1][PHI:PLO, :], t_dt[1][0:PHI, :])
            nc.vector.tensor_copy(t_u[1][PHI:PLO, :], t_u[1][0:PHI, :])

            # ---- write B/C planes to DRAM (section-major, coalescible) ----
            for r in range(2):
                src = t_proj[DTR + r * N:DTR + (r + 1) * N, :]
                nc.gpsimd.dma_start(
                    out=bass.AP(tensor=bc_lo.tensor,
                                offset=bc_lo.offset + r * N * SEC,
                                ap=[[SEC, N], [2 * N * SEC, NSEC], [1, SEC]]),
                    in_=bass.AP(tensor=src.tensor, offset=src.offset,
                                ap=[list(src.ap[0]), [SEC, NSEC], [1, SEC]]))

            # ================= phase B + C per section =================
            dma_engs = [nc.sync, nc.scalar]
            for si in range(NSEC):
                s0 = si * SEC
                sec_off = bc_lo.offset + si * 2 * N * SEC
                # lo broadcasts: 4 streaming groups of 4 items each
                t_blo = []
                for g in range(4):
                    tl = bc_pool.tile([PLO, 2, 4, SEC], f16, tag="blo",
                                      bufs=3, name=f"blo{g}_{si}")
                    dma_engs[g % 2].dma_start(
                        out=tl,
                        in_=bass.AP(tensor=bc_lo.tensor,
                                    offset=sec_off + 4 * g * SEC,
                                    ap=[[0, PLO], [N * SEC, 2], [SEC, 4],
                                        [1, SEC]]))
                    t_blo.append(tl)
                # hi broadcasts: 2 streaming groups of 4 pairs; even n rows
                # (k 0:8) -> top half, odd n rows (k 8:16) -> bottom half
                t_bhi = []
                for g in range(2):
                    th = bc_pool.tile([PLO, 2, 4, SEC], f16, tag="bhi",
                                      bufs=2, name=f"bhi{g}_{si}")
                    for par in range(2):
                        dma_engs[par].dma_start(
                            out=th[par * PHI:(par + 1) * PHI, :, :, :],
                            in_=bass.AP(tensor=bc_lo.tensor,
                                        offset=sec_off + (8 * par + 4 * g) * SEC,
                                        ap=[[0, PHI], [N * SEC, 2], [SEC, 4],
                                            [1, SEC]]))
                    t_bhi.append(th)

                ya_ps = [ya_ps_pool.tile([PLO, SEC], f32, tag="yaps0",
                                         name=f"yaps0_{si}"),
                         ya_ps_pool.tile([PHI, SEC], f32, tag="yaps1",
                                         name=f"yaps1_{si}")]
                # items: lo k ascending (matches streaming bcc groups),
                # hi pairs interleaved every two lo items
                items = []
                for j in range(N // 2):
                    items.append((0, 2 * j))
                    items.append((0, 2 * j + 1))
                    items.append((1, j))
                seen = [0, 0]
                ndve = [0, 0]
                for g, k in items:
                    first = seen[g] == 0
                    seen[g] += 1
                    last = seen[g] == (N if g == 0 else N // 2)
                    if g == 0:
                        bmb = t_blo[k // 4][:, 0, k % 4, :]
                        cmb = t_blo[k // 4][:, 1, k % 4, :]
                    else:
                        bmb = t_bhi[k // 4][:, 0, k % 4, :]
                        cmb = t_bhi[k // 4][:, 1, k % 4, :]
                    acol_t = t_aclo if g == 0 else t_achi
                    st_t = t_st[g][k]
                    dA = n_pool.tile([PLO, SEC], f16, tag="dA", bufs=2,
                                     name=f"dA_{si}_{g}_{k}")
                    nc.scalar.activation(out=dA, in_=t_dt[g][:, s0:s0 + SEC],
                                         func=Act.Exp,
                                         scale=acol_t[:, k:k + 1])
                    dBx = n_pool.tile([PLO, SEC], f16, tag="dBx", bufs=2,
                                      name=f"dBx_{si}_{g}_{k}")
                    use_dve_dbx = g == 0 and ndve[0] < n_dve_dbx
                    if use_dve_dbx:
                        ndve[0] += 1
                    (nc.vector if use_dve_dbx else nc.gpsimd).tensor_tensor(
                        out=dBx, in0=t_u[g][:, s0:s0 + SEC],
                        in1=bmb, op=Alu.mult)
                    h = n_pool.tile([PLO, SEC], f16, tag="h", bufs=3,
                                    name=f"h_{si}_{g}_{k}")
                    nc.vector.tensor_tensor_scan(
                        out=h, data0=dA, data1=dBx,
                        initial=(0.0 if si == 0 else st_t[:, 0:1]),
                        op0=Alu.mult, op1=Alu.add)
                    if si < NSEC - 1:
                        nc.scalar.activation(out=st_t[:, 0:1],
                                             in_=h[:, SEC - 1:SEC],
                                             func=Act.Copy)
                    hc = n_pool.tile([PLO, SEC], f16, tag="hc", bufs=2,
                                     name=f"hc_{si}_{g}_{k}")
                    use_dve_hc = (g == 0 and ndve[1] < n_dve_hc)
                    if use_dve_hc:
                        ndve[1] += 1
                    (nc.vector if use_dve_hc else nc.gpsimd).tensor_tensor(
                        out=hc, in0=h, in1=cmb, op=Alu.mult)
                    lhs = (t_idsel[:, 0:PLO] if g == 0
                           else t_idsel[:, PLO:PLO + PHI])
                    for q in range(SEC // MCH):
                        nc.tensor.matmul(
                            ya_ps[g][:, q * MCH:(q + 1) * MCH], lhs,
                            hc[:, q * MCH:(q + 1) * MCH],
                            start=first, stop=last)

                for q in range(SEC // MCH):
                    g0 = s0 + q * MCH
                    ps_o = ps_pool.tile([C, MCH], f32, tag="ps",
                                        name=f"pso_{si}_{q}")
                    for i in range(2):
                        pw = PW[i]
                        yg = tmp_pool.tile([PLO, MCH], f16, tag="yg",
                                           name=f"yg{i}_{si}_{q}")
                        nc.vector.scalar_tensor_tensor(
                            out=yg[:pw], in0=t_xa[i][:, g0:g0 + MCH],
                            scalar=t_dpv[i],
                            in1=ya_ps[i][:pw, q * MCH:(q + 1) * MCH],
                            op0=Alu.mult, op1=Alu.subtract)
                        ygz = tmp_pool.tile([PLO, MCH], f16, tag="ygz",
                                            name=f"ygz{i}_{si}_{q}")
                        nc.gpsimd.tensor_tensor(
                            out=ygz[:pw], in0=yg[:pw],
                            in1=t_zs[i][:, g0:g0 + MCH], op=Alu.mult)
                        nc.tensor.matmul(ps_o[:, :], t_woT[i], ygz[:pw, :],
                                         start=(i == 0), stop=(i == 1))
                    o_sb = tmp_pool.tile([C, MCH], f32, tag="osb",
                                         name=f"osb{si}_{q}")
                    nc.scalar.activation(out=o_sb, in_=ps_o, func=Act.Copy)
                    nc.gpsimd.dma_start(out=out[:, g0:g0 + MCH], in_=o_sb)

        if repeat == 1:
            body()
        else:
            with tc.For_i(0, repeat, 1) as iv:
                body(iv)

    nc.compile()
    return nc


def _prep_core_inputs(inp, d, seqT):
    W_in = inp['W_in'][d]
    conv_w = inp['conv_w'][d]
    A = np.exp(inp['A_log'][d])   # positive: dA = exp(dtm * (-A_true))
    wc = np.einsum('pc,pj->cjp', W_in[:P, :], conv_w)       # (C, DC, P)
    idsel = np.concatenate(
        [np.eye(PLO, dtype=np.float16),
         np.vstack([np.eye(PHI, dtype=np.float16)] * 2)], axis=1)
    # permute W_x columns (proj rows): [dtr | B NPERM | C NPERM]
    cols = (list(range(DTR)) + [DTR + k for k in NPERM]
            + [DTR + N + k for k in NPERM])
    wx_perm = inp['W_x'][d][cols, :]
    return {
        'seqT': np.ascontiguousarray(seqT).astype(np.float16),
        'wc': np.ascontiguousarray(wc).astype(np.float16),
        'wz': np.ascontiguousarray(W_in[P:, :].T).astype(np.float16),
        'wxT': np.ascontiguousarray(wx_perm.T).astype(np.float16),
        'wdtT': np.ascontiguousarray(inp['W_dt'][d].T).astype(np.float16),
        'woT': np.ascontiguousarray(inp['W_out'][d].T).astype(np.float16),
        'idsel': idsel,
        'bdt': np.ascontiguousarray(-inp['b_dt'][d][:, None], np.float32),
        'cb': np.ascontiguousarray(inp['conv_b'][d][:, None], np.float32),
        'dpv': np.ascontiguousarray(inp['Dp'][d][:, None], np.float32),
        'acol': np.ascontiguousarray(A, np.float32),
    }


def kernel(x1, x2, W_in, conv_w, conv_b, W_x, W_dt, b_dt, A_log, Dp, W_out):
    from concourse.bass_utils import run_bass_kernel_spmd

    inp = dict(x1=np.asarray(x1), x2=np.asarray(x2), W_in=np.asarray(W_in),
               conv_w=np.asarray(conv_w), conv_b=np.asarray(conv_b),
               W_x=np.asarray(W_x), W_dt=np.asarray(W_dt),
               b_dt=np.asarray(b_dt), A_log=np.asarray(A_log),
               Dp=np.asarray(Dp), W_out=np.asarray(W_out))
    B = inp['x1'].shape[0]

    if 'nc' not in _CACHED:
        _CACHED['nc'] = _build_program()
    nc = _CACHED['nc']

    in_maps = []
    metas = []
    for d in range(4):
        for b in range(B):
            x = inp['x1'][b]
            if d < 2:
                seq = x.reshape(C, L)
            else:
                seq = np.ascontiguousarray(x.transpose(0, 2, 1)).reshape(C, L)
            if d in (1, 3):
                seq = seq[:, ::-1]
            in_maps.append(_prep_core_inputs(inp, d, seq))
            metas.append((d, b))

    res = run_bass_kernel_spmd(nc, in_maps, core_ids=list(range(len(in_maps))))

    outs = np.zeros((B, C, HH, WW), np.float32)
    for (d, b), r in zip(metas, res.results):
        y = r['out']                      # (C, L)
        if d in (1, 3):
            y = y[:, ::-1]
        if d < 2:
            y = y.reshape(C, HH, WW)
        else:
            y = y.reshape(C, WW, HH).transpose(0, 2, 1)
        outs[b] += y
    outs += inp['x2']
    return outs


# revision 10
# speedup vs baseline: 100.1383x; 2.1752x over previous
"""Trainium2 Bass kernel for the 4-directional Mamba (SS2D / VMamba-style)
block from the OSS reference.

Sharding: the 8 independent (direction x batch) sequences map one-per-core
(SPMD: one NEFF, 8 cores, per-core inputs). Backward directions are handled by
host-side flips of the input/output sequences; the final sum of the four
directional outputs plus the residual x2 happens at gather time on host.

Per-core kernel (C=96, L=4096, P=192, N=16, dtr=6), v2:
  - causal depthwise conv folded into the input projection as 4 shifted
    tap-matmuls accumulating in PSUM (PE, fp16 operands)
  - silu / softplus computed directly by the ACT engine table functions
  - W_x output columns permuted host-side to [dtr | B even n | B odd n |
    C even | C odd] so the B/C planes written to DRAM are contiguous both
    for the lo-group (all 16 n) and the hi-group (even n -> top half,
    odd n -> bottom half) broadcast loads
  - B/C rows round-trip through DRAM in a section-major plane layout so the
    partition-broadcast loads coalesce into 2 lo-group + 2 hi-half DMAs per
    section with fat (16KB) descriptors — broadcast DMA cost on this HW is
    ~24ns/descriptor, so descriptor count is everything
  - the selective scan runs on the native DVE tensor_tensor_scan (f16, the
    fastest measured variant given the surrounding f16 2x-mode multiplies);
    P=192 states packed as a full 128-row group plus a pair-packed 64-row
    group
  - state hand-off between sections via 1-column ACT copies into st strips
  - dBx / hc multiplies split between DVE (f16 2x mode, ~0.8us/tile) and
    Pool (~2.4us/tile) to balance engine busy time; sum over n runs on the
    PE as identity-selection matmuls accumulating in PSUM
"""

import numpy as np

C = 96
L = 4096
P = 192
PLO = 128
PHI = 64
N = 16
DTR = 6
DC = 4
HH = 64
WW = 64
MCH = 512
NSEC = 4
SEC = L // NSEC

# lo-group item order: evens then odds (matches the permuted W_x columns)
NPERM = [2 * j for j in range(N // 2)] + [2 * j + 1 for j in range(N // 2)]

_CACHED = {}


def _build_program(repeat=1, n_dve_hc=4, n_dve_dbx=16):
    """n_dve_hc / n_dve_dbx: how many of the 16 lo hc / dBx mults per section
    run on DVE (the rest run on Pool) — engine balance knobs."""
    from contextlib import ExitStack

    import concourse.bacc as bacc
    import concourse.bass as bass
    import concourse.tile as tile
    from concourse import mybir

    f32 = mybir.dt.float32
    f16 = mybir.dt.float16
    Alu = mybir.AluOpType
    Act = mybir.ActivationFunctionType

    nc = bacc.Bacc()

    seqT = nc.dram_tensor("seqT", [C, L], f16, kind="ExternalInput")
    wc = nc.dram_tensor("wc", [C, DC, P], f16, kind="ExternalInput")
    wz = nc.dram_tensor("wz", [C, P], f16, kind="ExternalInput")
    wxT = nc.dram_tensor("wxT", [P, DTR + 2 * N], f16, kind="ExternalInput")
    wdtT = nc.dram_tensor("wdtT", [DTR, P], f16, kind="ExternalInput")
    woT = nc.dram_tensor("woT", [P, C], f16, kind="ExternalInput")
    idsel = nc.dram_tensor("idsel", [PLO, PLO + PHI], f16, kind="ExternalInput")
    bdt = nc.dram_tensor("bdt", [P, 1], f32, kind="ExternalInput")
    cb = nc.dram_tensor("cb", [P, 1], f32, kind="ExternalInput")
    dpv = nc.dram_tensor("dpv", [P, 1], f32, kind="ExternalInput")
    acol = nc.dram_tensor("acol", [P, N], f32, kind="ExternalInput")
    out = nc.dram_tensor("out", [C, L], f32, kind="ExternalOutput")

    with tile.TileContext(nc) as tc, ExitStack() as ctx:
        wpool = ctx.enter_context(tc.tile_pool(name="weights", bufs=1))
        spool = ctx.enter_context(tc.tile_pool(name="seq", bufs=1))
        big_pool = ctx.enter_context(tc.tile_pool(name="big", bufs=1))
        tmp_pool = ctx.enter_context(tc.tile_pool(name="tmp", bufs=2))
        n_pool = ctx.enter_context(tc.tile_pool(name="nl", bufs=3))
        bc_pool = ctx.enter_context(tc.tile_pool(name="bc", bufs=1))
        ps_pool = ctx.enter_context(tc.tile_pool(name="ps", bufs=3, space="PSUM"))
        ya_ps_pool = ctx.enter_context(
            tc.tile_pool(name="yps", bufs=1, space="PSUM"))
        dram_pool = ctx.enter_context(tc.tile_pool(name="dr", bufs=1, space="DRAM"))

        # ---- weights (lo = p 0:128, hi = p 128:192) ----
        t_wc = wpool.tile([C, DC, P], f16)
        t_wz = wpool.tile([C, P], f16)
        t_wxT = [wpool.tile([PLO, DTR + 2 * N], f16, name="wxT0"),
                 wpool.tile([PHI, DTR + 2 * N], f16, name="wxT1")]
        t_wdtT = wpool.tile([DTR, P], f16)
        t_woT = [wpool.tile([PLO, C], f16, name="woT0"),
                 wpool.tile([PHI, C], f16, name="woT1")]
        t_idsel = wpool.tile([PLO, PLO + PHI], f16)
        t_bdt = [wpool.tile([PLO, 1], f32, name="bdt0"),
                 wpool.tile([PHI, 1], f32, name="bdt1")]
        t_cb = [wpool.tile([PLO, 1], f32, name="cb0"),
                wpool.tile([PHI, 1], f32, name="cb1")]
        t_dpv = [wpool.tile([PLO, 1], f32, name="dpv0"),
                 wpool.tile([PHI, 1], f32, name="dpv1")]
        t_aclo = wpool.tile([PLO, N], f32)
        t_achi = wpool.tile([PLO, N // 2], f32)

        nc.sync.dma_start(out=t_wc, in_=wc[...])
        nc.sync.dma_start(out=t_wz, in_=wz[...])
        nc.sync.dma_start(out=t_wdtT, in_=wdtT[...])
        nc.sync.dma_start(out=t_idsel, in_=idsel[...])
        for i, (a, b) in enumerate([(0, PLO), (PLO, P)]):
            nc.sync.dma_start(out=t_wxT[i], in_=wxT[a:b, :])
            nc.sync.dma_start(out=t_woT[i], in_=woT[a:b, :])
            nc.sync.dma_start(out=t_bdt[i], in_=bdt[a:b, :])
            nc.sync.dma_start(out=t_cb[i], in_=cb[a:b, :])
            nc.sync.dma_start(out=t_dpv[i], in_=dpv[a:b, :])
        # lo A columns in NPERM order (matches lo item order)
        lo = acol[0:PLO, :]
        nc.sync.dma_start(
            out=t_aclo[:, 0:N // 2],
            in_=bass.AP(tensor=lo.tensor, offset=lo.offset,
                        ap=[list(lo.ap[0]), [2, N // 2]]))
        lo1 = acol[0:PLO, 1:]
        nc.sync.dma_start(
            out=t_aclo[:, N // 2:N],
            in_=bass.AP(tensor=lo1.tensor, offset=lo1.offset,
                        ap=[list(lo1.ap[0]), [2, N // 2]]))
        # hi pair-packed A columns: col j = [A[128:192, 2j]; A[128:192, 2j+1]]
        hi = acol[PLO:P, :]
        nc.sync.dma_start(
            out=t_achi[0:PHI, :],
            in_=bass.AP(tensor=hi.tensor, offset=hi.offset,
                        ap=[list(hi.ap[0]), [2, N // 2]]))
        hi1 = acol[PLO:P, 1:]
        nc.sync.dma_start(
            out=t_achi[PHI:PLO, :],
            in_=bass.AP(tensor=hi1.tensor, offset=hi1.offset,
                        ap=[list(hi1.ap[0]), [2, N // 2]]))

        # ---- padded sequence ----
        t_seq = spool.tile([C, L + DC - 1], f16)
        nc.vector.memset(t_seq[:, 0:DC - 1], 0.0)
        nc.sync.dma_start(out=t_seq[:, DC - 1:], in_=seqT[:, :])

        # persistent activations; hi-group dt/u replicated twice on partitions
        t_xa = [big_pool.tile([PLO, L], f16, name="xa0"),
                big_pool.tile([PHI, L], f16, name="xa1")]
        t_zs = [big_pool.tile([PLO, L], f16, name="zs0"),
                big_pool.tile([PHI, L], f16, name="zs1")]
        t_dt = [big_pool.tile([PLO, L], f16, name="dt0"),
                big_pool.tile([PLO, L], f16, name="dt1rep")]
        t_u = [big_pool.tile([PLO, L], f16, name="u0"),
               big_pool.tile([PLO, L], f16, name="u1rep")]
        t_proj = big_pool.tile([DTR + 2 * N, L], f16, name="proj")
        # per-item state columns (separate tiles so items don't serialize)
        t_st = [[big_pool.tile([PLO, 1], f32, name=f"stl{k}")
                 for k in range(N)],
                [big_pool.tile([PLO, 1], f32, name=f"sth{k}")
                 for k in range(N // 2)]]

        # DRAM staging: bc_lo[s, r, k, :] with r=0 -> B, r=1 -> C and k in
        # NPERM order (k<8: even n, k>=8: odd n), section-major.
        bc_lo = dram_pool.tile([NSEC, 2, N, SEC], f16)

        PW = [PLO, PHI]

        def body(_iv=None):
            # ================= phase A =================
            for s in range(L // MCH):
                g0 = s * MCH
                for i in range(2):
                    pw = PW[i]
                    ps_x = ps_pool.tile([PLO, MCH], f32, tag="ps",
                                        name=f"psx{i}_{s}")
                    for j in range(DC):
                        nc.tensor.matmul(ps_x[:pw, :],
                                         t_wc[:, j, i * PLO:i * PLO + pw],
                                         t_seq[:, g0 + j: g0 + j + MCH],
                                         start=(j == 0), stop=(j == DC - 1))
                    nc.scalar.activation(out=t_xa[i][:, g0:g0 + MCH],
                                         in_=ps_x[:pw], func=Act.Silu,
                                         bias=t_cb[i])

                    ps_z = ps_pool.tile([PLO, MCH], f32, tag="ps",
                                        name=f"psz{i}_{s}")
                    nc.tensor.matmul(ps_z[:pw, :],
                                     t_wz[:, i * PLO:i * PLO + pw],
                                     t_seq[:, g0 + DC - 1: g0 + DC - 1 + MCH],
                                     start=True, stop=True)
                    nc.scalar.activation(out=t_zs[i][:, g0:g0 + MCH],
                                         in_=ps_z[:pw], func=Act.Silu)

            for s in range(L // MCH):
                g0 = s * MCH
                ps_proj = ps_pool.tile([DTR + 2 * N, MCH], f32, tag="ps",
                                       name=f"psp_{s}")
                for i in range(2):
                    nc.tensor.matmul(ps_proj[:, :], t_wxT[i],
                                     t_xa[i][:, g0:g0 + MCH],
                                     start=(i == 0), stop=(i == 1))
                nc.scalar.activation(out=t_proj[:, g0:g0 + MCH], in_=ps_proj,
                                     func=Act.Copy)

            # dtm = -softplus(raw) built as ln(sigmoid(-raw)); staged through
            # t_u so the sigmoid and ln passes each load their table once
            for s in range(L // MCH):
                g0 = s * MCH
                for i in range(2):
                    pw = PW[i]
                    ps_dt = ps_pool.tile([PLO, MCH], f32, tag="ps",
                                         name=f"psdt{i}_{s}")
                    nc.tensor.matmul(ps_dt[:pw, :],
                                     t_wdtT[:, i * PLO:i * PLO + pw],
                                     t_proj[0:DTR, g0:g0 + MCH],
                                     start=True, stop=True)
                    nc.scalar.activation(out=t_u[i][:pw, g0:g0 + MCH],
                                         in_=ps_dt[:pw], func=Act.Sigmoid,
                                         scale=-1.0, bias=t_bdt[i])
            for i in range(2):
                pw = PW[i]
                nc.scalar.activation(out=t_dt[i][:pw], in_=t_u[i][:pw],
                                     func=Act.Ln)
                nc.vector.tensor_tensor(out=t_u[i][:pw], in0=t_dt[i][:pw],
                                        in1=t_xa[i][:, :], op=Alu.mult)
            # replicate hi rows [0:64] -> [64:128]
            nc.vector.tensor_copy(t_dt[1][PHI:PLO, :], t_dt[1][0:PHI, :])
            nc.vector.tensor_copy(t_u[1][PHI:PLO, :], t_u[1][0:PHI, :])

            # ---- write B/C planes to DRAM (section-major, coalescible) ----
            for r in range(2):
                src = t_proj[DTR + r * N:DTR + (r + 1) * N, :]
                nc.gpsimd.dma_start(
                    out=bass.AP(tensor=bc_lo.tensor,
                                offset=bc_lo.offset + r * N * SEC,
                                ap=[[SEC, N], [2 * N * SEC, NSEC], [1, SEC]]),
                    in_=bass.AP(tensor=src.tensor, offset=src.offset,
                                ap=[list(src.ap[0]), [SEC, NSEC], [1, SEC]]))

            # ================= phase B + C per section =================
            dma_engs = [nc.sync, nc.scalar]
            for si in range(NSEC):
                s0 = si * SEC
                sec_off = bc_lo.offset + si * 2 * N * SEC
                # lo broadcasts: 2 streaming groups of 8 items each
                # (fat 16KB descriptor runs, double-buffered across sections)
                t_blo = []
                for g in range(2):
                    tl = bc_pool.tile([PLO, 2, 8, SEC], f16, tag="blo",
                                      bufs=2, name=f"blo{g}_{si}")
                    dma_engs[g % 2].dma_start(
                        out=tl,
                        in_=bass.AP(tensor=bc_lo.tensor,
                                    offset=sec_off + 8 * g * SEC,
                                    ap=[[0, PLO], [N * SEC, 2], [SEC, 8],
                                        [1, SEC]]))
                    t_blo.append(tl)
                # hi broadcasts: 2 streaming groups of 4 pairs; even n rows
                # (k 0:8) -> top half, odd n rows (k 8:16) -> bottom half
                t_bhi = []
                for g in range(2):
                    th = bc_pool.tile([PLO, 2, 4, SEC], f16, tag="bhi",
                                      bufs=2, name=f"bhi{g}_{si}")
                    for par in range(2):
                        dma_engs[par].dma_start(
                            out=th[par * PHI:(par + 1) * PHI, :, :, :],
                            in_=bass.AP(tensor=bc_lo.tensor,
                                        offset=sec_off + (8 * par + 4 * g) * SEC,
                                        ap=[[0, PHI], [N * SEC, 2], [SEC, 4],
                                            [1, SEC]]))
                    t_bhi.append(th)

                ya_ps = [ya_ps_pool.tile([PLO, SEC], f32, tag="yaps0",
                                         name=f"yaps0_{si}"),
                         ya_ps_pool.tile([PHI, SEC], f32, tag="yaps1",
                                         name=f"yaps1_{si}")]
                # items: lo k ascending (matches streaming bcc groups),
                # hi pairs interleaved every two lo items
                items = []
                for j in range(N // 2):
                    items.append((0, 2 * j))
                    items.append((0, 2 * j + 1))
                    items.append((1, j))
                seen = [0, 0]
                ndve = [0, 0]
                for g, k in items:
                    first = seen[g] == 0
                    seen[g] += 1
                    last = seen[g] == (N if g == 0 else N // 2)
                    if g == 0:
                        bmb = t_blo[k // 8][:, 0, k % 8, :]
                        cmb = t_blo[k // 8][:, 1, k % 8, :]
                    else:
                        bmb = t_bhi[k // 4][:, 0, k % 4, :]
                        cmb = t_bhi[k // 4][:, 1, k % 4, :]
                    acol_t = t_aclo if g == 0 else t_achi
                    st_t = t_st[g][k]
                    dA = n_pool.tile([PLO, SEC], f16, tag="dA", bufs=2,
                                     name=f"dA_{si}_{g}_{k}")
                    nc.scalar.activation(out=dA, in_=t_dt[g][:, s0:s0 + SEC],
                                         func=Act.Exp,
                                         scale=acol_t[:, k:k + 1])
                    dBx = n_pool.tile([PLO, SEC], f16, tag="dBx", bufs=2,
                                      name=f"dBx_{si}_{g}_{k}")
                    use_dve_dbx = g == 0 and ndve[0] < n_dve_dbx
                    if use_dve_dbx:
                        ndve[0] += 1
                    (nc.vector if use_dve_dbx else nc.gpsimd).tensor_tensor(
                        out=dBx, in0=t_u[g][:, s0:s0 + SEC],
                        in1=bmb, op=Alu.mult)
                    h = n_pool.tile([PLO, SEC], f16, tag="h", bufs=3,
                                    name=f"h_{si}_{g}_{k}")
                    nc.vector.tensor_tensor_scan(
                        out=h, data0=dA, data1=dBx,
                        initial=(0.0 if si == 0 else st_t[:, 0:1]),
                        op0=Alu.mult, op1=Alu.add)
                    if si < NSEC - 1:
                        nc.scalar.activation(out=st_t[:, 0:1],
                                             in_=h[:, SEC - 1:SEC],
                                             func=Act.Copy)
                    hc = n_pool.tile([PLO, SEC], f16, tag="hc", bufs=2,
                                     name=f"hc_{si}_{g}_{k}")
                    use_dve_hc = (g == 0 and ndve[1] < n_dve_hc)
                    if use_dve_hc:
                        ndve[1] += 1
                    (nc.vector if use_dve_hc else nc.gpsimd).tensor_tensor(
                        out=hc, in0=h, in1=cmb, op=Alu.mult)
                    lhs = (t_idsel[:, 0:PLO] if g == 0
                           else t_idsel[:, PLO:PLO + PHI])
                    for q in range(SEC // MCH):
                        nc.tensor.matmul(
                            ya_ps[g][:, q * MCH:(q + 1) * MCH], lhs,
                            hc[:, q * MCH:(q + 1) * MCH],
                            start=first, stop=last)

                for q in range(SEC // MCH):
                    g0 = s0 + q * MCH
                    ps_o = ps_pool.tile([C, MCH], f32, tag="ps",
                                        name=f"pso_{si}_{q}")
                    for i in range(2):
                        pw = PW[i]
                        yg = tmp_pool.tile([PLO, MCH], f16, tag="yg",
                                           name=f"yg{i}_{si}_{q}")
                        nc.vector.scalar_tensor_tensor(
                            out=yg[:pw], in0=t_xa[i][:, g0:g0 + MCH],
                            scalar=t_dpv[i],
                            in1=ya_ps[i][:pw, q * MCH:(q + 1) * MCH],
                            op0=Alu.mult, op1=Alu.subtract)
                        ygz = tmp_pool.tile([PLO, MCH], f16, tag="ygz",
                                            name=f"ygz{i}_{si}_{q}")
                        nc.gpsimd.tensor_tensor(
                            out=ygz[:pw], in0=yg[:pw],
                            in1=t_zs[i][:, g0:g0 + MCH], op=Alu.mult)
                        nc.tensor.matmul(ps_o[:, :], t_woT[i], ygz[:pw, :],
                                         start=(i == 0), stop=(i == 1))
                    o_sb = tmp_pool.tile([C, MCH], f32, tag="osb",
                                         name=f"osb{si}_{q}")
                    nc.scalar.activation(out=o_sb, in_=ps_o, func=Act.Copy)
                    nc.gpsimd.dma_start(out=out[:, g0:g0 + MCH], in_=o_sb)

        if repeat == 1:
            body()
        else:
            with tc.For_i(0, repeat, 1) as iv:
                body(iv)

    nc.compile()
    return nc


def _prep_core_inputs(inp, d, seqT):
    W_in = inp['W_in'][d]
    conv_w = inp['conv_w'][d]
    A = np.exp(inp['A_log'][d])   # positive: dA = exp(dtm * (-A_true))
    wc = np.einsum('pc,pj->cjp', W_in[:P, :], conv_w)       # (C, DC, P)
    idsel = np.concatenate(
        [np.eye(PLO, dtype=np.float16),
         np.vstack([np.eye(PHI, dtype=np.float16)] * 2)], axis=1)
    # permute W_x columns (proj rows): [dtr | B NPERM | C NPERM]
    cols = (list(range(DTR)) + [DTR + k for k in NPERM]
            + [DTR + N + k for k in NPERM])
    wx_perm = inp['W_x'][d][cols, :]
    return {
        'seqT': np.ascontiguousarray(seqT).astype(np.float16),
        'wc': np.ascontiguousarray(wc).astype(np.float16),
        'wz': np.ascontiguousarray(W_in[P:, :].T).astype(np.float16),
        'wxT': np.ascontiguousarray(wx_perm.T).astype(np.float16),
        'wdtT': np.ascontiguousarray(inp['W_dt'][d].T).astype(np.float16),
        'woT': np.ascontiguousarray(inp['W_out'][d].T).astype(np.float16),
        'idsel': idsel,
        'bdt': np.ascontiguousarray(-inp['b_dt'][d][:, None], np.float32),
        'cb': np.ascontiguousarray(inp['conv_b'][d][:, None], np.float32),
        'dpv': np.ascontiguousarray(inp['Dp'][d][:, None], np.float32),
        'acol': np.ascontiguousarray(A, np.float32),
    }


def kernel(x1, x2, W_in, conv_w, conv_b, W_x, W_dt, b_dt, A_log, Dp, W_out):
    from concourse.bass_utils import run_bass_kernel_spmd

    inp = dict(x1=np.asarray(x1), x2=np.asarray(x2), W_in=np.asarray(W_in),
               conv_w=np.asarray(conv_w), conv_b=np.asarray(conv_b),
               W_x=np.asarray(W_x), W_dt=np.asarray(W_dt),
               b_dt=np.asarray(b_dt), A_log=np.asarray(A_log),
               Dp=np.asarray(Dp), W_out=np.asarray(W_out))
    B = inp['x1'].shape[0]

    if 'nc' not in _CACHED:
        _CACHED['nc'] = _build_program()
    nc = _CACHED['nc']

    in_maps = []
    metas = []
    for d in range(4):
        for b in range(B):
            x = inp['x1'][b]
            if d < 2:
                seq = x.reshape(C, L)
            else:
                seq = np.ascontiguousarray(x.transpose(0, 2, 1)).reshape(C, L)
            if d in (1, 3):
                seq = seq[:, ::-1]
            in_maps.append(_prep_core_inputs(inp, d, seq))
            metas.append((d, b))

    res = run_bass_kernel_spmd(nc, in_maps, core_ids=list(range(len(in_maps))))

    outs = np.zeros((B, C, HH, WW), np.float32)
    for (d, b), r in zip(metas, res.results):
        y = r['out']                      # (C, L)
        if d in (1, 3):
            y = y[:, ::-1]
        if d < 2:
            y = y.reshape(C, HH, WW)
        else:
            y = y.reshape(C, WW, HH).transpose(0, 2, 1)
        outs[b] += y
    outs += inp['x2']
    return outs
